# revision 43
# baseline (speedup 1.0000x reference)
"""Class-routed autoencoder (moe_routing) Trainium2 kernel.

Strategy:
- Route instead of computing all 10 experts densely: sort tokens by class on
  the host, split every class's tokens evenly across the 8 cores (padded up
  to a multiple of 8), so every core runs an IDENTICAL SPMD program on
  N_core = sum_e ceil(c_e/8) tokens laid out as 10 contiguous single-class
  segments. Expert layers slice the right weight block per segment at
  compile time; no gather/scatter on device.
- Everything runs feature-major ([features, tokens]): weights are the
  stationary matmul operand, the batch is the moving/free dim, and
  per-feature bias + ReLU + PSUM->SBUF evacuation fuse into one scalar (or
  vector) op. Matmuls in bf16 (fp32 PSUM accumulate).
- The kernel is PE-bound (448 matmul cycles/token ~ 192us @2.4GHz) with a
  ~37MB/core weight stream that nearly saturates HBM for the first ~110us.
  The schedule keeps both near-saturated:
  * encoder and decoder in three ~344-col chunks (fewest matmul
    instructions; per-instruction dispatch costs ~2-3ns), fused enc1+enc2
    and dec1+dec2 per chunk so the 2048-wide hiddens never leave SBUF.
  * W1/W2/x/dw1 are host-packed partition-major (4KB contiguous runs per
    partition — short strided runs cut DMA bandwidth several-fold) and
    m-sliced, so each enc1 PSUM only waits on its own m-slice: the
    DMA-paced start pipelines instead of blocking on all of W1.
  * Warmup matmuls on memset tiles keep the PE busy (and the p-state hot)
    during the initial DMA-paced window.
  * Expert weights stream on the gpsimd trigger queue, gated behind W2's
    arrival via a WAW dependency (a 1-elem copy into each ring tile that
    reads W2 — program order alone does NOT hold triggers back, the
    scheduler hoists them); 6-deep ring; ring-reuse pairs trigger at each
    expert's retirement.
  * Each expert is inserted one chunk AFTER its h2 segment completes, so
    its weights have arrived by the time the PE reaches it.
  * Decoder weights prefetch into SBUF freed by releasing the xc/W1 pool
    after the last enc1 matmul; their triggers are emitted after ALL
    expert-weight triggers so they never jump ahead on the wire, and the
    expert tail hides their latency.
  * The final chunk's output triggers spread across sync/scalar/gpsimd so
    the post-last-matmul drain is trigger-parallel.
- Host: permute+transpose x, run 8 cores, inverse-permute the output.
"""

import ml_dtypes
import numpy as np

import concourse.bass as bass
import concourse.mybir as mybir
import concourse.tile as tile
from concourse import bacc
from concourse.bass_utils import run_bass_kernel_spmd

N_CORES = 8
N_CLS = 10
D_IN, D_H, D_BOT, D_EXP = 1024, 2048, 512, 1024

F32 = mybir.dt.float32
BF16 = mybir.dt.bfloat16
RELU = mybir.ActivationFunctionType.Relu
IDENT = mybir.ActivationFunctionType.Identity

CHUNK = 344    # -> 3 enc column-chunks for n_core ~ 1026..1032
ECHUNK = 144   # expert unit width (segments are ~103-130, so 1 unit/expert)
EW_BUFS = 6    # expert-weight ring depth (pairs)
WARMUP = 40    # warmup matmuls (N=256) before real data lands

# packed-bias column offsets: [128, 164] f32
OB1, OB2, OEB1, OEB2, ODB1, ODB2, NBCOL = 0, 16, 20, 100, 140, 156, 164


def _chunks(n, step=CHUNK, s0=0):
    """Balanced split of n into ceil(n/step) near-equal pieces (all <= step)."""
    nch = -(-n // step)
    base, rem = divmod(n, nch)
    out = []
    s = s0
    for i in range(nch):
        sz = base + (1 if i < rem else 0)
        out.append((s, sz))
        s += sz
    return out


def _enc_chunks(n):
    """Near-equal chunks <= 344: three chunks -> fewest enc instructions while
    keeping expert-zipper insertion points spread through the encoder."""
    return _chunks(n)


def _dec_chunks(n):
    """Near-equal large chunks: output DMAs self-overlap with compute inside
    a large chunk, so only the final m-tile's trigger+transfer is exposed."""
    return _chunks(n)


def _build(n_seg, n_core):
    """Build the SPMD program for per-class-per-core counts n_seg (sum=n_core)."""
    nc = bacc.Bacc()

    chunks = _enc_chunks(n_core)
    # x and W1/W2 are host-packed partition-major so every DMA reads long
    # (>=4KB) contiguous runs per partition: short strided runs (~512B) cut
    # DMA bandwidth several-fold. One exactly-sized dram tensor per x chunk.
    xt = [nc.dram_tensor(f"xt{i}", [128, D_IN // 128, cl], BF16,
                         kind="ExternalInput")
          for i, (c0, cl) in enumerate(chunks)]
    w1 = nc.dram_tensor("w1", [8, 128, D_IN // 128, 256], BF16,
                        kind="ExternalInput")
    w2 = nc.dram_tensor("w2", [4, 128, D_H // 128, 128], BF16,
                        kind="ExternalInput")
    ew1 = nc.dram_tensor("ew1", [N_CLS, D_BOT, D_EXP], BF16, kind="ExternalInput")
    ew2 = nc.dram_tensor("ew2", [N_CLS, D_EXP, D_BOT], BF16, kind="ExternalInput")
    dw1 = nc.dram_tensor("dw1", [4, 128, D_BOT // 128, 512], BF16,
                        kind="ExternalInput")
    dw2 = nc.dram_tensor("dw2", [D_H, D_IN], BF16, kind="ExternalInput")
    bp = nc.dram_tensor("bp", [128, NBCOL], F32, kind="ExternalInput")
    # output in bf16: halves the output DMA drain; the rounding adds only
    # ~0.4% of each value, far under the 2e-2 gate (host converts to f32)
    out = nc.dram_tensor("out", [D_IN, n_core], BF16, kind="ExternalOutput")

    segs = []  # (class e, col start, col len)
    s = 0
    for e in range(N_CLS):
        if n_seg[e] > 0:
            segs.append((e, s, n_seg[e]))
            s += n_seg[e]

    KT1, MT1 = D_IN // 128, D_H // 128     # enc1: 8, 16
    KT2, MT2 = D_H // 128, D_BOT // 128    # enc2: 16, 4
    KE1, ME1 = D_BOT // 128, D_EXP // 128  # exp1: 4, 8
    KE2, ME2 = D_EXP // 128, D_BOT // 128  # exp2: 8, 4
    KD1, MD1 = D_BOT // 128, D_H // 128    # dec1: 4, 16
    KD2, MD2 = D_H // 128, D_IN // 128     # dec2: 16, 8

    with tile.TileContext(nc) as tc:
        p_const = tc.alloc_tile_pool(name="const", bufs=1)
        # PSUM tiles are full 2KB banks ([128,512] f32) so separate
        # accumulation groups never share a zero-region.
        p_ps = tc.alloc_tile_pool(name="ps", bufs=8, space="PSUM")

        def psum(cl):
            return p_ps.tile([128, 512], F32, tag="ps", name="ps")[:, :cl]

        # warmup tiles: memset by vector, matmul'd while DMAs stream in
        wu_w = p_const.tile([128, 128], BF16, tag="wu_w", name="wu_w")
        wu_x = p_const.tile([128, 256], BF16, tag="wu_x", name="wu_x")
        nc.vector.memset(wu_w[:], 0.0)
        nc.vector.memset(wu_x[:], 0.0)
        for _ in range(WARMUP):
            ps = p_ps.tile([128, 512], F32, tag="ps", name="ps")
            nc.tensor.matmul(ps[:, :256], wu_w[:], wu_x[:], start=True, stop=True)

        # single packed bias DMA (sync; right after xc0 below would be ideal,
        # but sync program order is: xc0, bias, w1 slices, w2 slices, xc1-3)
        bp_t = p_const.tile([128, NBCOL], F32, tag="bp", name="bp")

        # bottleneck activations, SBUF-resident at full width
        p_e2 = tc.alloc_tile_pool(name="e2", bufs=1)
        p_h2 = tc.alloc_tile_pool(name="h2", bufs=1)
        e2_t = [p_e2.tile([128, n_core], BF16, tag=f"e2_{m}", name=f"e2_{m}")
                for m in range(MT2)]
        h2_t = [p_h2.tile([128, n_core], BF16, tag=f"h2_{m}", name=f"h2_{m}")
                for m in range(MT2)]

        # Expert pool: allocated before the encoder pools so its space never
        # overlaps encoder tiles (no false deps on the weight ring DMAs).
        p_exp = tc.alloc_tile_pool(name="exp", bufs=1)
        e1_ring = [p_exp.tile([128, ME1, ECHUNK], BF16, tag=f"e1r_{i}",
                              name=f"e1r_{i}") for i in range(3)]
        ew1_ring = [p_exp.tile([128, KE1, D_EXP], BF16, tag=f"ew1_{i}",
                               name=f"ew1_{i}") for i in range(EW_BUFS)]
        ew2_ring = [p_exp.tile([128, KE2, D_BOT], BF16, tag=f"ew2_{i}",
                               name=f"ew2_{i}") for i in range(EW_BUFS)]

        # ---------------- encoder pools -----------------
        # p_enc1 (xc ring + W1) is released right after the last enc1 matmul
        # so decoder weights can prefetch into its space during the expert
        # tail. p_enc2 (W2 + h1c) is released after the last enc2 matmul.
        p_enc2 = tc.alloc_tile_pool(name="enc2", bufs=1, side="right")
        p_enc1 = tc.alloc_tile_pool(name="enc1", bufs=1)

        # All encoder-critical DMAs ride the sync queue in consumption order:
        # xc0, bias, W1 m-slices, W2 m-slices, then the remaining x chunks.
        # All chunks' xc tiles are live simultaneously (no ring reuse) so
        # every trigger fires up front.
        def load_xc(ci):
            t = p_enc1.tile([128, KT1, chunks[ci][1]], BF16, tag=f"xc{ci}",
                            name=f"xc{ci}")
            nc.sync.dma_start(out=t, in_=xt[ci][:])
            return t

        xcs = [load_xc(0)]
        nc.sync.dma_start(out=bp_t, in_=bp[:])

        # W1 m-sliced: 8 tiles of [128, KT1, 256] covering m-pairs, so the
        # first enc1 PSUM only waits for its own m-slice (pipelined start).
        w1_tiles = []
        for j in range(8):
            t = p_enc1.tile([128, KT1, 256], BF16, tag=f"w1_{j}", name=f"w1_{j}")
            nc.sync.dma_start(out=t, in_=w1[j])
            w1_tiles.append(t)

        def w1_at(k, m):
            return w1_tiles[m // 2][:, k, (m % 2) * 128:(m % 2 + 1) * 128]

        # W2 m-sliced: 4 tiles of [128, KT2, 128]
        w2_tiles = []
        for j in range(4):
            t = p_enc2.tile([128, KT2, 128], BF16, tag=f"w2_{j}", name=f"w2_{j}")
            nc.sync.dma_start(out=t, in_=w2[j])
            w2_tiles.append(t)

        def w2_at(k, m):
            return w2_tiles[m][:, k, :]

        for ci in range(1, len(chunks)):
            xcs.append(load_xc(ci))

        # Gate the expert-weight stream behind W2's arrival. Program order
        # alone does NOT hold back the dma triggers (the scheduler hoists
        # them past unrelated instructions), so create a real dependency:
        # write one element of every ring tile via a copy that READS the
        # last W2 tile — each ew DMA then carries a WAW dep on its tile and
        # cannot start before the encoder-critical bytes have landed.
        for rt in ew1_ring + ew2_ring:
            nc.gpsimd.tensor_copy(out=rt[:, 0, 0:1], in_=w2_tiles[3][:, 0, 0:1])

        def trig_ew(p):
            """Trigger the DMA pair for expert p into ring slot p%EW_BUFS."""
            e = segs[p][0]
            nc.gpsimd.dma_start(
                out=ew1_ring[p % EW_BUFS],
                in_=ew1[e].rearrange("(a p) n -> p a n", p=128))
            nc.gpsimd.dma_start(
                out=ew2_ring[p % EW_BUFS],
                in_=ew2[e].rearrange("(a p) n -> p a n", p=128))

        for p in range(min(EW_BUFS, len(segs))):
            trig_ew(p)

        # ---------------- expert machinery -----------------
        exp_counter = [0]
        unit_ctr = [0]
        pend = [None]  # exp2 of each unit is delayed one unit behind its exp1

        def emit_exp1(u):
            e, a, al, slot, ew1_t = u[:5]
            e1c = e1_ring[slot]
            for m in range(ME1):
                ps = psum(al)
                for k in range(KE1):
                    nc.tensor.matmul(ps, ew1_t[:, k, m * 128:(m + 1) * 128],
                                     h2_t[k][:, a:a + al],
                                     start=(k == 0), stop=(k == KE1 - 1))
                # bias+relu on the vector engine: keeps PSUM evacuation off
                # the scalar engine's critical path
                nc.vector.tensor_scalar(
                    out=e1c[:, m, :al], in0=ps,
                    scalar1=bp_t[:, OEB1 + e * ME1 + m:OEB1 + e * ME1 + m + 1],
                    scalar2=0.0,
                    op0=mybir.AluOpType.add, op1=mybir.AluOpType.max)

        def emit_exp2(u):
            e, a, al, slot, _, ew2_t = u[:6]
            e1c = e1_ring[slot]
            for m in range(ME2):
                ps = psum(al)
                for k in range(KE2):
                    nc.tensor.matmul(ps, ew2_t[:, k, m * 128:(m + 1) * 128],
                                     e1c[:, k, :al],
                                     start=(k == 0), stop=(k == KE2 - 1))
                nc.scalar.activation(
                    out=e2_t[m][:, a:a + al], in_=ps, func=RELU,
                    bias=bp_t[:, OEB2 + e * ME2 + m:OEB2 + e * ME2 + m + 1],
                    scale=1.0)

        def retire_pend():
            """Emit exp2 of the pending unit; when that unit completes its
            expert, its ring slot is fully read -> trigger the reuse pair."""
            u = pend[0]
            emit_exp2(u)
            pi, last = u[6], u[7]
            if last and pi + EW_BUFS < len(segs):
                trig_ew(pi + EW_BUFS)

        def emit_expert(pi):
            # exp1(unit i) then exp2(unit i-1): exp1's PSUM evacuations (DVE)
            # overlap the next unit's exp1 matmuls instead of stalling the PE
            e, s0, sl = segs[pi]
            ei = exp_counter[0]
            exp_counter[0] += 1
            ew1_t = ew1_ring[ei % EW_BUFS]
            ew2_t = ew2_ring[ei % EW_BUFS]
            echunks = _chunks(sl, ECHUNK)
            for ui, (c0, cl) in enumerate(echunks):
                u = (e, s0 + c0, cl, unit_ctr[0] % 3, ew1_t, ew2_t,
                     pi, ui == len(echunks) - 1)
                unit_ctr[0] += 1
                emit_exp1(u)
                if pend[0] is not None:
                    retire_pend()
                pend[0] = u

        # experts are inserted one chunk AFTER the chunk covering their
        # segment, so their weights (paced behind the encoder stream) have
        # arrived by the time the PE reaches them.
        seg_queue = list(range(len(segs)))

        def emit_ready_experts(covered_end):
            while seg_queue:
                pi = seg_queue[0]
                e, s0, sl = segs[pi]
                if s0 + sl <= covered_end:
                    seg_queue.pop(0)
                    emit_expert(pi)
                else:
                    break

        # ---------------- encoder (fused enc1+enc2 per chunk) -----------------
        mxe = max(cl for _, cl in chunks)
        h1c_t = [p_enc2.tile([128, mxe], BF16, tag=f"h1c_{m}", name=f"h1c_{m}")
                 for m in range(MT1)]

        for ci, (c0, cl) in enumerate(chunks):
            xc = xcs[ci]
            for m in range(MT1):
                ps = psum(cl)
                for k in range(KT1):
                    nc.tensor.matmul(ps, w1_at(k, m), xc[:, k, :cl],
                                     start=(k == 0), stop=(k == KT1 - 1))
                nc.scalar.activation(out=h1c_t[m][:, :cl], in_=ps, func=RELU,
                                     bias=bp_t[:, OB1 + m:OB1 + m + 1],
                                     scale=1.0)
            if ci == len(chunks) - 1:
                # xc + W1 are fully consumed once these matmuls issue: free
                # their space for the decoder weights (tiles reserved here;
                # the DMA triggers are emitted only after ALL expert-weight
                # triggers so they never jump ahead of them on the wire).
                p_enc1.release()
                p_dec1 = tc.alloc_tile_pool(name="dec1", bufs=1)
                dw1_tiles = []
                for j in range(4):
                    t = p_dec1.tile([128, KD1, 512], BF16, tag=f"dw1_{j}",
                                    name=f"dw1_{j}")
                    dw1_tiles.append(t)
                dw2_a = p_dec1.tile([128, KD2 // 2, D_IN], BF16, tag="dw2a",
                                    name="dw2a")
            for m in range(MT2):
                ps = psum(cl)
                for k in range(KT2):
                    nc.tensor.matmul(ps, w2_at(k, m), h1c_t[k][:, :cl],
                                     start=(k == 0), stop=(k == KT2 - 1))
                nc.scalar.activation(out=h2_t[m][:, c0:c0 + cl], in_=ps,
                                     func=RELU,
                                     bias=bp_t[:, OB2 + m:OB2 + m + 1],
                                     scale=1.0)
            # insert experts covered by the PREVIOUS chunk (one-chunk delay)
            if ci > 0:
                emit_ready_experts(chunks[ci - 1][0] + chunks[ci - 1][1])

        p_enc2.release()
        p_dec2 = tc.alloc_tile_pool(name="dec2", bufs=1, side="right")
        dw2_b = p_dec2.tile([128, KD2 // 2, D_IN], BF16, tag="dw2b", name="dw2b")

        def dw2_at(k):
            return (dw2_a if k < 8 else dw2_b)[:, k % 8, :]

        # remaining experts (their h2 segments completed in the last chunks)
        while seg_queue:
            emit_expert(seg_queue.pop(0))
        if pend[0] is not None:
            retire_pend()
            pend[0] = None

        # decoder-weight prefetch: emitted after every ew trigger so the
        # expert stream keeps wire priority; the expert tail hides these.
        for j in range(4):
            nc.gpsimd.dma_start(out=dw1_tiles[j], in_=dw1[j])
        nc.gpsimd.dma_start(
            out=dw2_a,
            in_=dw2[:8 * 128, :].rearrange("(a p) n -> p a n", p=128))
        nc.gpsimd.dma_start(
            out=dw2_b,
            in_=dw2[8 * 128:, :].rearrange("(a p) n -> p a n", p=128))

        # ---------------- decoder (fused dec1+dec2 per chunk) -----------------
        dchunks = _dec_chunks(n_core)
        mxd = max(cl for _, cl in dchunks)
        d1c_t = [p_dec2.tile([128, mxd], BF16, tag=f"d1c_{m}", name=f"d1c_{m}")
                 for m in range(MD1)]
        for c0, cl in dchunks:
            for m in range(MD1):
                ps = psum(cl)
                for k in range(KD1):
                    nc.tensor.matmul(
                        ps, dw1_tiles[m // 4][:, k,
                                              (m % 4) * 128:(m % 4 + 1) * 128],
                        e2_t[k][:, c0:c0 + cl],
                        start=(k == 0), stop=(k == KD1 - 1))
                nc.scalar.activation(out=d1c_t[m][:, :cl], in_=ps, func=RELU,
                                     bias=bp_t[:, ODB1 + m:ODB1 + m + 1],
                                     scale=1.0)
            for m in range(MD2):
                ps = psum(cl)
                for k in range(KD2):
                    nc.tensor.matmul(ps, dw2_at(k)[:, m * 128:(m + 1) * 128],
                                     d1c_t[k][:, :cl],
                                     start=(k == 0), stop=(k == KD2 - 1))
                o_t = p_dec2.tile([128, mxd], BF16, tag="o", name="o",
                                  bufs=4)
                nc.scalar.activation(out=o_t[:, :cl], in_=ps, func=IDENT,
                                     bias=bp_t[:, ODB2 + m:ODB2 + m + 1],
                                     scale=1.0)
                if (c0, cl) == dchunks[-1]:
                    # final chunk: spread triggers so the post-last-matmul
                    # drain is trigger-parallel, not serialized on sync
                    eng = (nc.sync, nc.scalar, nc.gpsimd)[m % 3]
                else:
                    eng = nc.sync
                eng.dma_start(
                    out=out[m * 128:(m + 1) * 128, c0:c0 + cl],
                    in_=o_t[:, :cl])

        p_dec2.release()
        p_dec1.release()
        p_exp.release()
        p_h2.release()
        p_e2.release()
        p_ps.release()
        p_const.release()

    nc.finalize()
    return nc


_CACHE = {}


def _get_nc(n_seg, n_core):
    key = tuple(n_seg)
    if key not in _CACHE:
        _CACHE[key] = _build(n_seg, n_core)
    return _CACHE[key]


def _bf16(a):
    return np.ascontiguousarray(np.asarray(a, np.float32).astype(ml_dtypes.bfloat16))


def _pack_biases(b1, b2, Eb1, Eb2, Db1, Db2):
    bp = np.zeros((128, NBCOL), np.float32)
    bp[:, OB1:OB1 + 16] = np.asarray(b1, np.float32).reshape(16, 128).T
    bp[:, OB2:OB2 + 4] = np.asarray(b2, np.float32).reshape(4, 128).T
    bp[:, OEB1:OEB1 + 80] = (np.asarray(Eb1, np.float32)
                             .reshape(N_CLS, 8, 128).transpose(2, 0, 1)
                             .reshape(128, 80))
    bp[:, OEB2:OEB2 + 40] = (np.asarray(Eb2, np.float32)
                             .reshape(N_CLS, 4, 128).transpose(2, 0, 1)
                             .reshape(128, 40))
    bp[:, ODB1:ODB1 + 16] = np.asarray(Db1, np.float32).reshape(16, 128).T
    bp[:, ODB2:ODB2 + 8] = np.asarray(Db2, np.float32).reshape(8, 128).T
    return np.ascontiguousarray(bp)


def kernel(x, labels, W1, b1, W2, b2, EW1, Eb1, EW2, Eb2, DW1, Db1, DW2, Db2):
    x = np.asarray(x, dtype=np.float32)
    labels_np = np.asarray(labels).astype(np.int64)
    B = x.shape[0]

    counts = np.bincount(labels_np, minlength=N_CLS)
    n_seg = [int(-(-int(c) // N_CORES)) for c in counts]  # ceil(c/8)
    n_core = int(sum(n_seg))

    # assign tokens: class e sorted tokens padded to 8*n_seg[e], row j -> core j
    order = np.argsort(labels_np, kind="stable")
    idx_by_class = np.split(order, np.cumsum(counts)[:-1])
    core_tok = np.full((N_CORES, n_core), -1, dtype=np.int64)
    off = 0
    for e in range(N_CLS):
        ne = n_seg[e]
        if ne == 0:
            continue
        padded = np.full(N_CORES * ne, -1, dtype=np.int64)
        padded[:counts[e]] = idx_by_class[e]
        core_tok[:, off:off + ne] = padded.reshape(N_CORES, ne)
        off += ne

    # partition-major packed weights: w1 [8,128,8,256], w2 [4,128,16,128]
    w1p = np.ascontiguousarray(
        _bf16(W1).reshape(8, 128, 8, 256).transpose(2, 1, 0, 3))
    w2p = np.ascontiguousarray(
        _bf16(W2).reshape(16, 128, 4, 128).transpose(2, 1, 0, 3))
    dw1p = np.ascontiguousarray(
        _bf16(DW1).reshape(4, 128, 4, 512).transpose(2, 1, 0, 3))
    weights = {
        "w1": w1p, "w2": w2p,
        "ew1": _bf16(EW1), "ew2": _bf16(EW2),
        "dw1": dw1p, "dw2": _bf16(DW2),
        "bp": _pack_biases(b1, b2, Eb1, Eb2, Db1, Db2),
    }

    chunks = _enc_chunks(n_core)
    x_bf = x.astype(ml_dtypes.bfloat16)
    in_maps = []
    for j in range(N_CORES):
        ids = core_tok[j]
        valid = ids >= 0
        xc = np.zeros((n_core, D_IN), dtype=ml_dtypes.bfloat16)
        xc[valid] = x_bf[ids[valid]]
        # pack x per chunk, partition-major: xt{i} [128, 8, cl]
        x3 = np.ascontiguousarray(xc.T).reshape(8, 128, n_core)
        im = {}
        for ci, (c0, cl) in enumerate(chunks):
            im[f"xt{ci}"] = np.ascontiguousarray(
                x3[:, :, c0:c0 + cl].transpose(1, 0, 2))
        im.update(weights)
        in_maps.append(im)

    nc = _get_nc(n_seg, n_core)
    res = run_bass_kernel_spmd(nc, in_maps, core_ids=list(range(N_CORES)))

    out = np.empty((B, D_IN), dtype=np.float32)
    for j in range(N_CORES):
        oc = np.asarray(res.results[j]["out"], dtype=np.float32)
        ids = core_tok[j]
        valid = ids >= 0
        out[ids[valid]] = oc.T[valid]
    return out
